# revision 20
# baseline (speedup 1.0000x reference)
"""Trainium2 Bass kernel for BlockdiagButterflyLinear.

Computes y = butterfly(x; w1, w2) + bias where
  tmp[b,k,j,y] = sum_i x[b, k*1024+i] * w1[k, j*48+y, i]
  out[b, 4l+j] = sum_{k,y} tmp[b,k,j,y] * w2[j, l, k*48+y] + bias[4l+j]

Sharding: data-parallel over the 8192 token rows across 8 NeuronCores
(1024 tokens/core); the small butterfly factors are replicated.

All device I/O is fp16 (host converts; the 2e-2 rel-err budget dwarfs
fp16 quantization), halving HBM traffic vs fp32.  x is transposed on
the host to feature-major layout so the device needs no PE transposes:
stage-1 reads x^T tiles [i, token] straight from DRAM.

Per-core pipeline (four 256-token chunks):
  1. DMA x^T in [128 i, 8 ic, 256 tok] tiles (one per (chunk, k)).
  2. Stage 1: per (k, jt) one 8-step accumulation of [112, 256] in PSUM,
     rows = [j_even y0:48 | pad | j_odd y0:48] with j = 2*jt + {0,1}.
     lhsT = resident w1 slices [128 i, 112], moving = x^T.
  3. Copies PSUM->SBUF build stage-2 lhsT tiles t2[j][c] of 113
     partitions: rows 0:48 = k=2c, rows 64:112 = k=2c+1, zero gap rows,
     constant-one row 112 (bias folded into stage-2 weights).  All
     partition starts are in {0, 32, 64, 96} per the engine rule.
  4. Stage 2 matmuls: out[tok, l] accumulating the two 113-row ky
     chunks per (m, j, lc); result copied with stride-4 interleave into
     the fp16 output tile (l*4+j feature order).
  5. DMA out [128, 2048] halves as soon as their copies land.
"""

import sys

sys.path.insert(0, "/opt/trn_rl_repo")

from contextlib import ExitStack

import numpy as np

import concourse.bacc as bacc
import concourse.bass as bass
import concourse.mybir as mybir
import concourse.tile as tile
from concourse.bass_utils import run_bass_kernel_spmd
from concourse.masks import make_identity

F16 = mybir.dt.float16
F32 = mybir.dt.float32

N_CORES = 8
TOK_PER_CORE = 1024  # 8192 tokens / 8 cores
N_FEAT = 4096
K, J, B1 = 4, 4, 48
CH = 256  # token chunk
MB = CH // 128  # 128-token subchunks per chunk
N_CH = TOK_PER_CORE // CH
L = 1024  # l dim of stage 2 per j
R2 = 113  # stage-2 contraction rows: 48 + 16 gap + 48 + 1 ones row
W1C = 224  # w1 columns per k: 2 jt blocks of [48 | 16 pad | 48]

_PROGRAM = None


def _build_program() -> bass.Bass:
    nc = bacc.Bacc(None, target_bir_lowering=False)
    xs = nc.declare_dram_parameter("xs", [K, 128, 8, TOK_PER_CORE], F16, isOutput=False)
    w1t = nc.declare_dram_parameter("w1t", [K, 128, 8, W1C], F16, isOutput=False)
    w2t = nc.declare_dram_parameter("w2t", [J, R2, 2, L], F16, isOutput=False)
    out = nc.declare_dram_parameter("out", [TOK_PER_CORE, N_FEAT], F16, isOutput=True)

    with ExitStack() as ctx:
        tc = ctx.enter_context(tile.TileContext(nc))
        consts = ctx.enter_context(tc.tile_pool(name="consts", bufs=1))
        wpool = ctx.enter_context(tc.tile_pool(name="wpool", bufs=1))
        xpool = ctx.enter_context(tc.tile_pool(name="xpool", bufs=16))
        outpool = ctx.enter_context(tc.tile_pool(name="outpool", bufs=5))
        p1pool = ctx.enter_context(tc.tile_pool(name="p1pool", bufs=2, space="PSUM"))
        p2pool = ctx.enter_context(tc.tile_pool(name="p2pool", bufs=5, space="PSUM"))
        pwpool = ctx.enter_context(tc.tile_pool(name="pwpool", bufs=1, space="PSUM"))

        # fp32 scratch operand + PSUM bank for PE-warmup junk matmuls: the
        # cost model runs the PE at 1.2 GHz until it has been continuously
        # busy for 3 us, so burn that ramp on matmuls that depend only on
        # engine-generated data (no DMA) while the first x tiles stream in.
        wsrc = consts.tile([128, 128], F32)
        nc.any.memset(wsrc[:], 0.0)
        pwarm = pwpool.tile([128, 128], F32)

        def warm(n):
            for _ in range(n):
                nc.tensor.matmul(pwarm[:], wsrc[:], wsrc[:], start=True, stop=True)

        w1sk = [wpool.tile([128, 8, W1C], F16, name=f"w1s_{k}") for k in range(K)]
        w2sj = [wpool.tile([R2, 2, L], F16, name=f"w2s_{j}") for j in range(J)]

        def load_w1(k):
            nc.sync.dma_start(w1sk[k][:], w1t[k])

        def load_w2(j, lc):
            # lc-half loads: stage-2(0) m0 consumes lc=0 slices first, so
            # all four lc=0 halves land before any lc=1 bytes.  Rows 48:64
            # are never loaded - the matching t2 rows are hard zeros.
            for r0, r1 in ((0, 48), (64, R2)):
                nc.sync.dma_start(
                    w2sj[j][r0:r1, :, lc * 512 : (lc + 1) * 512],
                    w2t[j][r0:r1, :, lc * 512 : (lc + 1) * 512],
                )

        # Stage-2 lhsT tiles, statically double-buffered by chunk parity:
        # rows 0:48 = (k=2c), 64:112 = (k=2c+1), 112 = ones.  The zero gap
        # rows 48:64 / ones row are initialized once per physical tile; the
        # per-chunk copies only rewrite rows 0:48 and 64:112, so rows 48:64
        # stay zero and row 112 stays one (rows 32:48 / 96:112 of the inits
        # are overwritten by the first chunk's copies - memset partition
        # starts must be 32-aligned).
        t2s = [
            [
                [
                    consts.tile([R2, CH], F16, name=f"t2_{j}_{c}_{par}")
                    for par in range(2)
                ]
                for c in range(2)
            ]
            for j in range(J)
        ]
        for j in range(J):
            for c in range(2):
                for par in range(2):
                    nc.any.memset(t2s[j][c][par][32:64, :], 0.0)
                    nc.any.memset(t2s[j][c][par][96:R2, :], 1.0)

        xtiles = {}

        def load_x(ch, k):
            xm = xpool.tile([128, 8, CH], F16, tag="xk", name=f"x_{ch}_{k}")
            nc.sync.dma_start(xm[:], xs[k][:, :, ch * CH : (ch + 1) * CH])
            xtiles[(ch, k)] = xm

        # Issue ALL input DMAs upfront (xpool holds all 16 x tiles, so no
        # WAR hazards).  Out-DMAs emitted later wait on copy sems while
        # holding the SP sequencer, so anything queued after them would
        # stall - inputs must all be ahead of the first out-DMA.  The
        # stream order tracks PE consumption: w1/x0 interleaved, x1, w2
        # (lc-halves, consumed by st2(0)), x2, x3.
        # k0 in interleaved ic-halves so the first matmuls start ~2.8us in
        xm0 = xpool.tile([128, 8, CH], F16, tag="xk", name="x_0_0")
        xtiles[(0, 0)] = xm0
        nc.sync.dma_start(w1sk[0][:, 0:4, :], w1t[0][:, 0:4, :])
        nc.sync.dma_start(xm0[:, 0:4, :], xs[0][:, 0:4, 0:CH])
        nc.sync.dma_start(w1sk[0][:, 4:8, :], w1t[0][:, 4:8, :])
        nc.sync.dma_start(xm0[:, 4:8, :], xs[0][:, 4:8, 0:CH])
        for k in range(1, K):
            load_w1(k)
            load_x(0, k)
        for k in range(K):
            load_x(1, k)
        for lc in range(2):
            for j in range(J):
                load_w2(j, lc)
        for ch in range(2, N_CH):
            for k in range(K):
                load_x(ch, k)

        cp_ctr = [0]

        def copy(dst, src):
            # alternate PSUM->SBUF copies between DVE and ACT
            if cp_ctr[0] % 2 == 0:
                nc.vector.tensor_copy(dst, src)
            else:
                nc.scalar.copy(dst, src)
            cp_ctr[0] += 1

        def stage1(ch, junk=(0, 0, 0, 0)):
            par = ch % 2
            for k in range(K):
                xk = xtiles[(ch, k)]
                p1 = p1pool.tile([112, 2 * CH], F32, tag="p1")
                for jt in range(2):
                    for ic in range(8):
                        nc.tensor.matmul(
                            p1[:, jt * CH : (jt + 1) * CH],
                            w1sk[k][:, ic, jt * 112 : (jt + 1) * 112],
                            xk[:, ic, :],
                            start=(ic == 0),
                            stop=(ic == 7),
                        )
                warm(junk[k])
                for jt in range(2):
                    for jj in range(2):
                        j = 2 * jt + jj
                        copy(
                            t2s[j][k // 2][par][
                                (k % 2) * 64 : (k % 2) * 64 + 48, :
                            ],
                            p1[jj * 64 : jj * 64 + 48, jt * CH : (jt + 1) * CH],
                        )

        def stage2(ch):
            par = ch % 2
            last = ch == N_CH - 1
            for m in range(MB):
                outm = outpool.tile([128, L, 4], F16, tag="outm")
                row0 = ch * CH + m * 128
                # quarter-granular MM/copy/store chains on the last m-tile so
                # the final store waits only on one [128, 256] copy leg
                nq = 2 if (last and m == MB - 1) else 1
                qw = 512 // nq
                for lc in range(2):
                    for q in range(nq):
                        for j in range(J):
                            p2 = p2pool.tile([128, 512], F32, tag="p2")
                            for c in range(2):
                                nc.tensor.matmul(
                                    p2[:, 0:qw],
                                    t2s[j][c][par][:, m * 128 : (m + 1) * 128],
                                    w2sj[j][
                                        :, c, lc * 512 + q * qw : lc * 512 + (q + 1) * qw
                                    ],
                                    start=(c == 0),
                                    stop=(c == 1),
                                )
                            copy(
                                outm[
                                    :, lc * 512 + q * qw : lc * 512 + (q + 1) * qw, j
                                ],
                                p2[:, 0:qw],
                            )
                        # each piece stores as soon as its four j copies land
                        nc.sync.dma_start(
                            out[
                                row0 : row0 + 128,
                                (lc * 512 + q * qw) * 4 : (lc * 512 + (q + 1) * qw) * 4,
                            ],
                            outm[:, lc * 512 + q * qw : lc * 512 + (q + 1) * qw, :],
                        )

        # PE order: warmup, st1(0) st1(1) st2(0) st1(2) st2(1) st2(2)
        # st1(3) st2(3).  st2(2) runs before st1(3) (its t2 par-0 inputs
        # come from st1(2)) so its stores fill the DMA idle window and only
        # one st2 chunk's stores trail the final matmuls.  t2 parity WARs
        # stay legal: st1(2) overwrites par0 after st2(0)'s matmuls, and
        # st1(3) overwrites par1 after st2(1)'s.
        warm(6)
        stage1(0)
        stage1(1)
        stage2(0)
        stage1(2)
        stage2(1)
        stage2(2)
        stage1(3)
        stage2(3)

    nc.compile()
    nc.finalize()
    return nc


def _get_program() -> bass.Bass:
    global _PROGRAM
    if _PROGRAM is None:
        _PROGRAM = _build_program()
    return _PROGRAM


def _prep_weights(w1, w2, b):
    # w1t[k, p, ic, jt*112 + jj*64 + y] = w1[k, (2*jt+jj)*48 + y, ic*128 + p]
    w1r = (
        w1.transpose(0, 2, 1)
        .astype(np.float16)
        .reshape(K, 8, 128, 4, 48)  # [k, ic, p, j, y]
    )
    w1p = np.zeros((K, 8, 128, 2, 112), np.float16)
    w1p[:, :, :, :, 0:48] = w1r[:, :, :, 0::2, :]
    w1p[:, :, :, :, 64:112] = w1r[:, :, :, 1::2, :]
    w1t = np.ascontiguousarray(
        w1p.transpose(0, 2, 1, 3, 4).reshape(K, 128, 8, W1C)
    )

    # w2t[j, r, c, l]: rows 0:48 = k=2c, 64:112 = k=2c+1, 112 = bias (c=1)
    w2r = w2.transpose(0, 2, 1).astype(np.float16)  # [j, kb1, l]
    w2t = np.zeros((J, R2, 2, L), np.float16)
    for c in range(2):
        w2t[:, 0:48, c, :] = w2r[:, (2 * c) * 48 : (2 * c) * 48 + 48, :]
        w2t[:, 64:112, c, :] = w2r[:, (2 * c + 1) * 48 : (2 * c + 1) * 48 + 48, :]
    for j in range(J):
        w2t[j, 112, 1, :] = b[j::J].astype(np.float16)  # bias[4l+j]
    return w1t, w2t


def kernel(x, w1_bfly, w2_bfly, bias):
    x = np.asarray(x, dtype=np.float32)
    w1 = np.asarray(w1_bfly, dtype=np.float32)
    w2 = np.asarray(w2_bfly, dtype=np.float32)
    b = np.asarray(bias, dtype=np.float32)

    x_shape = x.shape
    # xh[c, k, p, ic, t] = x[c*1024 + t, k*1024 + ic*128 + p], fp16
    xh = (
        x.reshape(N_CORES, TOK_PER_CORE, K, 8, 128)
        .transpose(0, 2, 4, 3, 1)
        .astype(np.float16, order="C")
    )
    w1t, w2t = _prep_weights(w1, w2, b)

    nc = _get_program()
    in_maps = [
        {"xs": xh[c], "w1t": w1t, "w2t": w2t}
        for c in range(N_CORES)
    ]
    res = run_bass_kernel_spmd(nc, in_maps, core_ids=list(range(N_CORES)))
    outs = [np.asarray(res.results[c]["out"]) for c in range(N_CORES)]
    full = np.concatenate(outs, axis=0).astype(np.float32)
    return full.reshape(x_shape[:-1] + (N_FEAT,))


# revision 23
# speedup vs baseline: 1.0065x; 1.0065x over previous
"""Trainium2 Bass kernel for BlockdiagButterflyLinear.

Computes y = butterfly(x; w1, w2) + bias where
  tmp[b,k,j,y] = sum_i x[b, k*1024+i] * w1[k, j*48+y, i]
  out[b, 4l+j] = sum_{k,y} tmp[b,k,j,y] * w2[j, l, k*48+y] + bias[4l+j]

Sharding: data-parallel over the 8192 token rows across 8 NeuronCores
(1024 tokens/core); the small butterfly factors are replicated.

All device I/O is fp16 (host converts; the 2e-2 rel-err budget dwarfs
fp16 quantization), halving HBM traffic vs fp32.  x is transposed on
the host to feature-major layout so the device needs no PE transposes:
stage-1 reads x^T tiles [i, token] straight from DRAM.

Per-core pipeline (four 256-token chunks):
  1. DMA x^T in [128 i, 8 ic, 256 tok] tiles (one per (chunk, k)).
  2. Stage 1: per (k, jt) one 8-step accumulation of [112, 256] in PSUM,
     rows = [j_even y0:48 | pad | j_odd y0:48] with j = 2*jt + {0,1}.
     lhsT = resident w1 slices [128 i, 112], moving = x^T.
  3. Copies PSUM->SBUF build stage-2 lhsT tiles t2[j][c] of 113
     partitions: rows 0:48 = k=2c, rows 64:112 = k=2c+1, zero gap rows,
     constant-one row 112 (bias folded into stage-2 weights).  All
     partition starts are in {0, 32, 64, 96} per the engine rule.
  4. Stage 2 matmuls: out[tok, l] accumulating the two 113-row ky
     chunks per (m, j, lc); result copied with stride-4 interleave into
     the fp16 output tile (l*4+j feature order).
  5. DMA out [128, 2048] halves as soon as their copies land.
"""

import sys

sys.path.insert(0, "/opt/trn_rl_repo")

from contextlib import ExitStack

import numpy as np

import concourse.bacc as bacc
import concourse.bass as bass
import concourse.mybir as mybir
import concourse.tile as tile
from concourse.bass_utils import run_bass_kernel_spmd
from concourse.masks import make_identity

F16 = mybir.dt.float16
F32 = mybir.dt.float32

N_CORES = 8
TOK_PER_CORE = 1024  # 8192 tokens / 8 cores
N_FEAT = 4096
K, J, B1 = 4, 4, 48
CH = 256  # token chunk
MB = CH // 128  # 128-token subchunks per chunk
N_CH = TOK_PER_CORE // CH
L = 1024  # l dim of stage 2 per j
R2 = 113  # stage-2 contraction rows: 48 + 16 gap + 48 + 1 ones row
W1C = 224  # w1 columns per k: 2 jt blocks of [48 | 16 pad | 48]

_PROGRAM = None


def _build_program() -> bass.Bass:
    nc = bacc.Bacc(None, target_bir_lowering=False)
    xs = nc.declare_dram_parameter("xs", [K, 128, 8, TOK_PER_CORE], F16, isOutput=False)
    w1t = nc.declare_dram_parameter("w1t", [K, 128, 8, W1C], F16, isOutput=False)
    w2t = nc.declare_dram_parameter("w2t", [J, R2, 2, L], F16, isOutput=False)
    out = nc.declare_dram_parameter("out", [TOK_PER_CORE, N_FEAT], F16, isOutput=True)

    with ExitStack() as ctx:
        tc = ctx.enter_context(tile.TileContext(nc))
        consts = ctx.enter_context(tc.tile_pool(name="consts", bufs=1))
        wpool = ctx.enter_context(tc.tile_pool(name="wpool", bufs=1))
        xpool = ctx.enter_context(tc.tile_pool(name="xpool", bufs=16))
        outpool = ctx.enter_context(tc.tile_pool(name="outpool", bufs=5))
        p1pool = ctx.enter_context(tc.tile_pool(name="p1pool", bufs=2, space="PSUM"))
        p2pool = ctx.enter_context(tc.tile_pool(name="p2pool", bufs=5, space="PSUM"))
        pwpool = ctx.enter_context(tc.tile_pool(name="pwpool", bufs=1, space="PSUM"))

        # fp32 scratch operand + PSUM bank for PE-warmup junk matmuls: the
        # cost model runs the PE at 1.2 GHz until it has been continuously
        # busy for 3 us, so burn that ramp on matmuls that depend only on
        # engine-generated data (no DMA) while the first x tiles stream in.
        wsrc = consts.tile([128, 128], F32)
        nc.any.memset(wsrc[:], 0.0)
        pwarm = pwpool.tile([128, 128], F32)

        def warm(n):
            for _ in range(n):
                nc.tensor.matmul(pwarm[:], wsrc[:], wsrc[:], start=True, stop=True)

        w1sk = [wpool.tile([128, 8, W1C], F16, name=f"w1s_{k}") for k in range(K)]
        w2sj = [wpool.tile([R2, 2, L], F16, name=f"w2s_{j}") for j in range(J)]

        def load_w1(k):
            nc.sync.dma_start(w1sk[k][:], w1t[k])

        def load_w2(j, lc):
            # lc-half loads: stage-2(0) m0 consumes lc=0 slices first, so
            # all four lc=0 halves land before any lc=1 bytes.  Rows 48:64
            # are never loaded - the matching t2 rows are hard zeros.
            for r0, r1 in ((0, 48), (64, R2)):
                nc.sync.dma_start(
                    w2sj[j][r0:r1, :, lc * 512 : (lc + 1) * 512],
                    w2t[j][r0:r1, :, lc * 512 : (lc + 1) * 512],
                )

        # Stage-2 lhsT tiles, statically double-buffered by chunk parity:
        # rows 0:48 = (k=2c), 64:112 = (k=2c+1), 112 = ones.  The zero gap
        # rows 48:64 / ones row are initialized once per physical tile; the
        # per-chunk copies only rewrite rows 0:48 and 64:112, so rows 48:64
        # stay zero and row 112 stays one (rows 32:48 / 96:112 of the inits
        # are overwritten by the first chunk's copies - memset partition
        # starts must be 32-aligned).
        t2s = [
            [
                [
                    consts.tile([R2, CH], F16, name=f"t2_{j}_{c}_{par}")
                    for par in range(2)
                ]
                for c in range(2)
            ]
            for j in range(J)
        ]
        for j in range(J):
            for c in range(2):
                for par in range(2):
                    nc.any.memset(t2s[j][c][par][32:64, :], 0.0)
                    nc.any.memset(t2s[j][c][par][96:R2, :], 1.0)

        xtiles = {}

        def load_x(ch, k, split=False):
            xm = xpool.tile([128, 8, CH], F16, tag="xk", name=f"x_{ch}_{k}")
            src = xs[k][:, :, ch * CH : (ch + 1) * CH]
            if split:
                # ic-half loads: the k-segment's first 8 matmuls only need
                # ic 0:4, so the PE starts ~0.7us before the full tile lands
                nc.sync.dma_start(xm[:, 0:4, :], src[:, 0:4, :])
                nc.sync.dma_start(xm[:, 4:8, :], src[:, 4:8, :])
            else:
                nc.sync.dma_start(xm[:], src)
            xtiles[(ch, k)] = xm

        # Issue ALL input DMAs upfront (xpool holds all 16 x tiles, so no
        # WAR hazards).  Out-DMAs emitted later wait on copy sems while
        # holding the SP sequencer, so anything queued after them would
        # stall - inputs must all be ahead of the first out-DMA.  The
        # stream order tracks PE consumption: w1/x0 interleaved, x1, w2
        # (lc-halves, consumed by st2(0)), x2, x3.
        # chunk-0/1 tiles stream as interleaved ic-halves (w1a xa w1b xb)
        # so every DMA-gated k-segment starts on its first half
        for k in range(K):
            xm = xpool.tile([128, 8, CH], F16, tag="xk", name=f"x_0_{k}")
            xtiles[(0, k)] = xm
            nc.sync.dma_start(w1sk[k][:, 0:4, :], w1t[k][:, 0:4, :])
            nc.sync.dma_start(xm[:, 0:4, :], xs[k][:, 0:4, 0:CH])
            nc.sync.dma_start(w1sk[k][:, 4:8, :], w1t[k][:, 4:8, :])
            nc.sync.dma_start(xm[:, 4:8, :], xs[k][:, 4:8, 0:CH])
        for k in range(K):
            load_x(1, k, split=True)
        for lc in range(2):
            for j in range(J):
                load_w2(j, lc)
        for ch in range(2, N_CH):
            for k in range(K):
                load_x(ch, k)

        cp_ctr = [0]

        def copy(dst, src):
            # alternate PSUM->SBUF copies between DVE and ACT
            if cp_ctr[0] % 2 == 0:
                nc.vector.tensor_copy(dst, src)
            else:
                nc.scalar.copy(dst, src)
            cp_ctr[0] += 1

        def stage1(ch, junk=(0, 0, 0, 0)):
            par = ch % 2
            for k in range(K):
                xk = xtiles[(ch, k)]
                p1 = p1pool.tile([112, 2 * CH], F32, tag="p1")
                for jt in range(2):
                    for ic in range(8):
                        nc.tensor.matmul(
                            p1[:, jt * CH : (jt + 1) * CH],
                            w1sk[k][:, ic, jt * 112 : (jt + 1) * 112],
                            xk[:, ic, :],
                            start=(ic == 0),
                            stop=(ic == 7),
                        )
                warm(junk[k])
                for jt in range(2):
                    for jj in range(2):
                        j = 2 * jt + jj
                        copy(
                            t2s[j][k // 2][par][
                                (k % 2) * 64 : (k % 2) * 64 + 48, :
                            ],
                            p1[jj * 64 : jj * 64 + 48, jt * CH : (jt + 1) * CH],
                        )

        def stage2(ch):
            par = ch % 2
            last = ch == N_CH - 1
            for m in range(MB):
                outm = outpool.tile([128, L, 4], F16, tag="outm")
                row0 = ch * CH + m * 128
                # quarter-granular MM/copy/store chains on the last m-tile so
                # the final store waits only on one [128, 256] copy leg
                nq = 2 if (last and m == MB - 1) else 1
                qw = 512 // nq
                for lc in range(2):
                    for q in range(nq):
                        for j in range(J):
                            p2 = p2pool.tile([128, 512], F32, tag="p2")
                            for c in range(2):
                                nc.tensor.matmul(
                                    p2[:, 0:qw],
                                    t2s[j][c][par][:, m * 128 : (m + 1) * 128],
                                    w2sj[j][
                                        :, c, lc * 512 + q * qw : lc * 512 + (q + 1) * qw
                                    ],
                                    start=(c == 0),
                                    stop=(c == 1),
                                )
                            copy(
                                outm[
                                    :, lc * 512 + q * qw : lc * 512 + (q + 1) * qw, j
                                ],
                                p2[:, 0:qw],
                            )
                        # each piece stores as soon as its four j copies land
                        nc.sync.dma_start(
                            out[
                                row0 : row0 + 128,
                                (lc * 512 + q * qw) * 4 : (lc * 512 + (q + 1) * qw) * 4,
                            ],
                            outm[:, lc * 512 + q * qw : lc * 512 + (q + 1) * qw, :],
                        )

        # PE order: warmup, st1(0) st1(1) st2(0) st1(2) st2(1) st2(2)
        # st1(3) st2(3).  st2(2) runs before st1(3) (its t2 par-0 inputs
        # come from st1(2)) so its stores fill the DMA idle window and only
        # one st2 chunk's stores trail the final matmuls.  t2 parity WARs
        # stay legal: st1(2) overwrites par0 after st2(0)'s matmuls, and
        # st1(3) overwrites par1 after st2(1)'s.
        warm(8)
        stage1(0)
        stage1(1)
        stage2(0)
        stage1(2)
        stage2(1)
        stage1(3)
        stage2(2)
        stage2(3)

    nc.compile()
    nc.finalize()
    return nc


def _get_program() -> bass.Bass:
    global _PROGRAM
    if _PROGRAM is None:
        _PROGRAM = _build_program()
    return _PROGRAM


def _prep_weights(w1, w2, b):
    # w1t[k, p, ic, jt*112 + jj*64 + y] = w1[k, (2*jt+jj)*48 + y, ic*128 + p]
    w1r = (
        w1.transpose(0, 2, 1)
        .astype(np.float16)
        .reshape(K, 8, 128, 4, 48)  # [k, ic, p, j, y]
    )
    w1p = np.zeros((K, 8, 128, 2, 112), np.float16)
    w1p[:, :, :, :, 0:48] = w1r[:, :, :, 0::2, :]
    w1p[:, :, :, :, 64:112] = w1r[:, :, :, 1::2, :]
    w1t = np.ascontiguousarray(
        w1p.transpose(0, 2, 1, 3, 4).reshape(K, 128, 8, W1C)
    )

    # w2t[j, r, c, l]: rows 0:48 = k=2c, 64:112 = k=2c+1, 112 = bias (c=1)
    w2r = w2.transpose(0, 2, 1).astype(np.float16)  # [j, kb1, l]
    w2t = np.zeros((J, R2, 2, L), np.float16)
    for c in range(2):
        w2t[:, 0:48, c, :] = w2r[:, (2 * c) * 48 : (2 * c) * 48 + 48, :]
        w2t[:, 64:112, c, :] = w2r[:, (2 * c + 1) * 48 : (2 * c + 1) * 48 + 48, :]
    for j in range(J):
        w2t[j, 112, 1, :] = b[j::J].astype(np.float16)  # bias[4l+j]
    return w1t, w2t


def kernel(x, w1_bfly, w2_bfly, bias):
    x = np.asarray(x, dtype=np.float32)
    w1 = np.asarray(w1_bfly, dtype=np.float32)
    w2 = np.asarray(w2_bfly, dtype=np.float32)
    b = np.asarray(bias, dtype=np.float32)

    x_shape = x.shape
    # xh[c, k, p, ic, t] = x[c*1024 + t, k*1024 + ic*128 + p], fp16
    xh = (
        x.reshape(N_CORES, TOK_PER_CORE, K, 8, 128)
        .transpose(0, 2, 4, 3, 1)
        .astype(np.float16, order="C")
    )
    w1t, w2t = _prep_weights(w1, w2, b)

    nc = _get_program()
    in_maps = [
        {"xs": xh[c], "w1t": w1t, "w2t": w2t}
        for c in range(N_CORES)
    ]
    res = run_bass_kernel_spmd(nc, in_maps, core_ids=list(range(N_CORES)))
    outs = [np.asarray(res.results[c]["out"]) for c in range(N_CORES)]
    full = np.concatenate(outs, axis=0).astype(np.float32)
    return full.reshape(x_shape[:-1] + (N_FEAT,))
